# revision 2
# baseline (speedup 1.0000x reference)
"""Trainium2 Bass kernel for BinaryCE + rejection-softmax loss.

Reference computation (B=256, C=500, D=256):
    y = labels.astype(f32)                                   # [B, C]
    bce[b] = sum_c( softplus(logits) - y*logits )            # log-sigmoid BCE
    max_sim[b, c] = max_d wf[c, b, d]
    rej[b] = sum_c (labels==0) * relu(sigmoid(max_sim) - 0.3)
    out[b] = bce[b] + rej[b]

Sharding: data-parallel over B across 8 cores (wf on axis 1,
logits/labels on axis 0). Per core: logits [32,500], wf [500,32,256],
labels [32,500] -> out [32]. No cross-device reduction.

Layout: the wf slice is viewed as [125 partitions, 32768] with
partition p holding the 4 consecutive classes c = 4p..4p+3 - each
partition reads one fully contiguous 128 KB run (the only DMA shape
that streams at full HBM rate; measured ~410 GB/s single-core on the
SWDGE queue). Trace-verified budget of the previous 64.1us baseline:
~6us runtime preamble, ~3us SWDGE spin-up, 40.9us stream (zero gaps),
then a ~4.3us DVE catch-up + ~5.5us tail. This version attacks the
non-stream parts:
  * no 512-class padding (125 partitions exact): -2.3% HBM bytes
  * wf is cast f32->fp16 during the DMA (SWDGE inline cast), so the
    DVE reduce_max runs 16-bit perf modes and never lags the stream
  * chunks are c4-slab aligned with a small head chunk (the first
    descgen finishes sooner -> stream starts ~1.5us earlier); every
    chunk has its own SBUF buffer (fp16 halves the footprint), so Q7
    descgen never waits on compute
  * the last slab's reduce + rejection chain is split so only a
    2048-elem reduce + a narrow 8-column chain remains after the
    final byte lands
BCE on the ACT ring and the label-mask transposes (PE) run entirely
under the stream. Per-class sums collapse through a ones-vector
matmul into PSUM [1, 32] with the BCE column injected via an
identity-matmul transpose.
"""

import sys

for _p in ("/root/.axon_site", "/root/.axon_site/_ro/trn_rl_repo",
           "/root/.axon_site/_ro/pypackages", "/opt/trn_rl_repo"):
    if _p not in sys.path:
        sys.path.append(_p)

import numpy as np

import concourse.bass as bass  # noqa: F401  (registers engine classes)
import concourse.tile as tile
from concourse import bacc, mybir
from concourse.bass_utils import run_bass_kernel_spmd
from concourse.masks import make_identity

F32 = mybir.dt.float32
F16 = mybir.dt.float16
I32 = mybir.dt.int32
AF = mybir.ActivationFunctionType
ALU = mybir.AluOpType
AX = mybir.AxisListType

B, C, D = 256, 500, 256
REJECTION_MARGIN = 0.3
NCORES = 8
BL = B // NCORES          # 32 samples per core
C4 = 4                    # classes per partition
NP = C // C4              # 125 partitions, no padding
SLAB = BL * D             # 8192 elems per (partition, c4)

WF_DT = F16               # wf is cast f32->fp16 during the DMA

# (elem offset per partition, length, c4, first b) - c4-slab aligned,
# small head chunk, small final chunk.
CHUNKS = [
    (0,         2048, 0, 0),
    (2048,      6144, 0, 8),
    (SLAB,      SLAB, 1, 0),
    (2 * SLAB,  SLAB, 2, 0),
    (3 * SLAB,  6144, 3, 0),
    (3 * SLAB + 6144, 2048, 3, 24),
]


def build_nc(debug: bool = False):
    nc = bacc.Bacc("TRN2", target_bir_lowering=False, debug=debug)

    logits_d = nc.dram_tensor("logits", [BL, C], F32, kind="ExternalInput")
    wf_d = nc.dram_tensor("wf", [C, BL, D], F32, kind="ExternalInput")
    labels_d = nc.dram_tensor("labels", [BL, C], I32, kind="ExternalInput")
    out_d = nc.dram_tensor("out", [1, BL], F32, kind="ExternalOutput")

    # [125, 32768]: partition p = classes 4p..4p+3, contiguous per partition
    wfv = wf_d[:].rearrange("(p c4) b d -> p (c4 b d)", c4=C4)

    with tile.TileContext(nc) as tc:
        with (
            tc.tile_pool(name="consts", bufs=1) as consts,
            tc.tile_pool(name="psum_t", bufs=2, space="PSUM") as psum_t,
            tc.tile_pool(name="psum_acc", bufs=1, space="PSUM") as psum_acc,
        ):
            # --- wf stream: all descgens first on the Q7, distinct
            # buffers so nothing ever waits on compute ------------------
            wfts = []
            for off, ln, _c4, _b0 in CHUNKS:
                wft = consts.tile([NP, ln], WF_DT)
                nc.gpsimd.dma_start(wft[:], wfv[:, off:off + ln])
                wfts.append(wft)

            # --- small inputs on the ACT ring (tiny, independent) -------
            logits_sb = consts.tile([BL, C], F32)
            nc.scalar.dma_start(logits_sb[:], logits_d[:])
            labels_sb = consts.tile([BL, C], I32)
            nc.scalar.dma_start(labels_sb[:], labels_d[:])

            # identity after the descgens: gpsimd program order would
            # otherwise delay the first wf chunk by the Q7 launches.
            ident = consts.tile([BL, BL], F32)
            make_identity(nc, ident[:])

            labels_f = consts.tile([BL, C], F32)
            nc.vector.tensor_copy(labels_f[:], labels_sb[:])

            ones = consts.tile([NP, 1], F32)
            nc.vector.memset(ones[:], 1.0)
            neg_margin = consts.tile([NP, 1], F32)
            nc.vector.memset(neg_margin[:], -REJECTION_MARGIN)

            # --- BCE part in natural [b, c] layout -------------------------
            # softplus(x) = ln(exp(x) + 1); no Softplus LUT on TRN2.
            # Safe: |logits| <~ 5 so exp() cannot overflow.
            exp_tmp = consts.tile([BL, C], F32)
            nc.scalar.activation(exp_tmp[:], logits_sb[:], AF.Exp)
            sp_tmp = consts.tile([BL, C], F32)
            sp_sum = consts.tile([BL, 1], F32)
            nc.scalar.activation(sp_tmp[:], exp_tmp[:], AF.Ln, bias=1.0,
                                 accum_out=sp_sum[:])
            yx_tmp = consts.tile([BL, C], F32)
            yx_sum = consts.tile([BL, 1], F32)
            nc.vector.tensor_mul(yx_tmp[:], labels_f[:], logits_sb[:])
            nc.vector.reduce_sum(yx_sum[:], yx_tmp[:], axis=AX.X)
            bce_col = consts.tile([BL, 1], F32)
            nc.vector.tensor_sub(bce_col[:], sp_sum[:], yx_sum[:])

            # --- mask = 1 - labels^T in [p, c4, b] layout (c = 4p + c4) ----
            mask_sb = consts.tile([NP, C4, BL], F32)
            for c4 in range(C4):
                labT = psum_t.tile([NP, BL], F32, tag="labT")
                nc.tensor.matmul(labT[:], labels_f[:, c4::C4], ident[:],
                                 start=True, stop=True)
                nc.scalar.activation(mask_sb[:, c4, :], labT[:],
                                     AF.Identity, bias=1.0, scale=-1.0)

            # --- PSUM accumulator [1, 32]; BCE row first -------------------
            acc = psum_acc.tile([1, BL], F32)
            nc.tensor.matmul(acc[:], bce_col[:], ident[:],
                             start=True, stop=False)

            # --- stream reduces + masked rejection chains ------------------
            msim = consts.tile([NP, C4, BL], WF_DT)

            def chain(c4, b0, nb, stop):
                sl = slice(b0, b0 + nb)
                sig = consts.tile([NP, nb], F32)
                nc.scalar.activation(sig[:], msim[:, c4, sl], AF.Sigmoid)
                rej = consts.tile([NP, nb], F32)
                nc.scalar.activation(rej[:], sig[:], AF.Relu,
                                     bias=neg_margin[:])
                rejm = consts.tile([NP, nb], F32)
                nc.vector.tensor_mul(rejm[:], rej[:], mask_sb[:, c4, sl])
                nc.tensor.matmul(acc[:, sl], ones[:], rejm[:],
                                 start=False, stop=stop)

            for i, (off, ln, c4, b0) in enumerate(CHUNKS):
                nb = ln // D
                nc.vector.reduce_max(
                    msim[:, c4, b0:b0 + nb],
                    wfts[i][:].rearrange("p (b d) -> p b d", d=D), axis=AX.X)
                if c4 < C4 - 1:
                    if b0 + nb == BL:      # slab complete -> full chain
                        chain(c4, 0, BL, stop=False)
                else:
                    # last slab: per-chunk chains so only an 8-column
                    # chain remains after the final byte lands
                    chain(c4, b0, nb, stop=True)

            out_sb = consts.tile([1, BL], F32)
            nc.scalar.copy(out_sb[:], acc[:])
            nc.scalar.dma_start(out_d[:], out_sb[:])

    nc.compile()
    return nc


_NC_CACHE = None


def _get_nc():
    global _NC_CACHE
    if _NC_CACHE is None:
        _NC_CACHE = build_nc()
    return _NC_CACHE


def _in_maps(logits, wf, labels):
    maps = []
    for k in range(NCORES):
        b0 = k * BL
        maps.append({
            "logits": np.ascontiguousarray(logits[b0:b0 + BL]),
            "wf": np.ascontiguousarray(wf[:, b0:b0 + BL, :]),
            "labels": np.ascontiguousarray(labels[b0:b0 + BL]),
        })
    return maps


def run(logits, wf, labels, trace: bool = False, tmpdir: str | None = None):
    """Run on all 8 cores; returns (full_output [B], BassKernelResults)."""
    logits = np.asarray(logits, dtype=np.float32)
    wf = np.asarray(wf, dtype=np.float32)
    labels = np.asarray(labels, dtype=np.int32)
    assert logits.shape == (B, C) and wf.shape == (C, B, D) \
        and labels.shape == (B, C)

    nc = _get_nc()
    res = run_bass_kernel_spmd(nc, _in_maps(logits, wf, labels),
                               list(range(NCORES)), trace=trace,
                               tmpdir=tmpdir)
    out = np.concatenate(
        [np.asarray(res.results[k]["out"]).reshape(BL) for k in range(NCORES)])
    return out.astype(np.float32), res


def kernel(logits, wf, labels):
    out, _ = run(logits, wf, labels)
    return out
